# revision 32
# baseline (speedup 1.0000x reference)
"""MoE group-limited routing gate (DeepSeek-style) on 8 Trainium2 NeuronCores.

Computation (per token t over E=256 experts, D=7168 features):
    logits = x @ weight.T                      [T, E]
    group-limited top-k: 8 groups of 32 experts, keep top-4 groups by
    group-max, then top-8 experts among kept groups.
    weights = sigmoid(logits[sel]) normalized to sum 1, * 2.5
Returns (weights [T,8] f32, indices [T,8] int32) like the reference.

Strategy: data-parallel over tokens, 2048 tokens/core, gate weight
replicated.  Matmul precision "fp16fp8":
    logits = xh16 @ wh16  +  2^-17 * (x8 @ wl8 + xl8 @ wh8)
  - main pass: fp16 (11-bit significands, products exact in f32 PSUM)
  - correction: ONE fp8e4m3 DoubleRow pass fusing both residual terms
    (x8 = fp8(xh) cast on-device; xl8 = fp8((x-xh16)*2^11) from host;
    wl8 = fp8((w-wh16)*2^17); wh8 = fp8(w*2^6); descale 2^-17).
    DoubleRow runs fp8 at 2x rate, so the whole correction costs one
    bf16-rate pass -> 2 pass-equivalents total vs fp16x3's 3.
  Host-measured logit err ~1.8e-5 -> idx rel-err ~6e-3 on the graded
  inputs (vs 2e-2 gate).
DMA: x rides the SP HWDGE ring at 3 B/elem (xh16 2B + xl8 1B) in
partition-major contiguous blocks; the small replicated weights ride
the Activation HWDGE ring in parallel so they never serialize the x
stream.  Outputs ride the gpsimd SWDGE ring.
Top-k uses the DVE native max/max_index (top-8 sorted) instructions;
the group top-4 uses a threshold trick (4th-largest group-max) since
sigmoid is monotone and masking is additive on logits.
"""

import numpy as np
import ml_dtypes
from contextlib import ExitStack

import concourse.bacc as bacc
import concourse.tile as tile
from concourse import mybir
from concourse.bass_utils import run_bass_kernel_spmd

N_CORES = 8
T_FULL = 16384
D = 7168
E = 256
G = 8            # expert groups
EPG = E // G     # experts per group = 32
TOPK = 8
TOPK_GROUPS = 4
ROUTE_SCALE = 2.5

P = 128
T = T_FULL // N_CORES       # 2048 tokens per core
KC = D // P                 # 56 contraction chunks
TB = 256                    # tokens per block
NB = T // TB                # 8 blocks
TPB = TB // P               # 2 token-tiles per block
KQ = 4                      # x DMA splits per block
KCQ = KC // KQ              # 14 k-chunks per split
WQ = 8                      # weight DMA splits
WCQ = KC // WQ              # 7 k-chunks per split
NEG = -1.0e30
XL_SCALE = 2.0 ** 11        # xl8 = fp8((x - xh16) * XL_SCALE)
W_SCALE = 2.0 ** 17         # wh16 stores w * 2^17 (fits fp16, no subnormals).
                            # Then main (xh@wh16) and correction (x8@wl8 +
                            # xl8@wh8) are BOTH at scale 2^17, so the fp8
                            # correction accumulates directly into the main
                            # PSUM tile; 2^-17 folds into the sigmoid scale.
WH8_DESCALE = 1.0 / XL_SCALE  # wh8 = fp8(wh16 * 2^-11), on-device cast
E4 = ml_dtypes.float8_e4m3
PRECISION = "fp16fp8"       # "fp16fp8" | "fp16x3"

_CACHE = {}


def _emit_topk(nc, sc_pool, out_pool, scores, oout, t0,
               sig_scale=1.0, out_eng=None):
    """Group-limited top-k + normalize on a [128, 256] f32 logits tile.

    ``scores`` may be pre-scaled logits (monotone, so group-mask and top-k
    are unaffected); ``sig_scale`` restores true logits inside the sigmoid.
    Writes one merged [P, 2, TOPK] u32 output row-block: pair 0 = weights
    (f32 bitcast), pair 1 = indices.
    """
    f32 = mybir.dt.float32
    scores_g = scores.rearrange("p (g e) -> p g e", g=G)
    glog = sc_pool.tile([P, G], f32)
    nc.vector.reduce_max(out=glog, in_=scores_g, axis=mybir.AxisListType.X)
    gsort = sc_pool.tile([P, G], f32)
    nc.vector.max(out=gsort, in_=glog)
    # additive mask: 0 for kept groups (>= 4th-largest), -1e30 otherwise
    maskadd = sc_pool.tile([P, G], f32)
    nc.vector.tensor_scalar(
        out=maskadd,
        in0=glog,
        scalar1=gsort[:, TOPK_GROUPS - 1:TOPK_GROUPS],
        scalar2=NEG,
        op0=mybir.AluOpType.is_lt,
        op1=mybir.AluOpType.mult,
    )
    masked = sc_pool.tile([P, E], f32)
    nc.vector.tensor_add(
        masked.rearrange("p (g e) -> p g e", g=G),
        scores_g,
        maskadd.to_broadcast([P, G, EPG]),
    )
    top8 = sc_pool.tile([P, TOPK], f32)
    nc.vector.max(out=top8, in_=masked)
    comb = out_pool.tile([P, 2, TOPK], mybir.dt.uint32)
    nc.vector.max_index(out=comb[:, 1, :], in_max=top8, in_values=masked)
    sig = sc_pool.tile([P, TOPK], f32)
    nc.scalar.activation(
        out=sig, in_=top8, func=mybir.ActivationFunctionType.Sigmoid,
        scale=sig_scale,
    )
    ssum = sc_pool.tile([P, 1], f32)
    nc.vector.reduce_sum(out=ssum, in_=sig, axis=mybir.AxisListType.X)
    rec = sc_pool.tile([P, 1], f32)
    nc.vector.reciprocal(out=rec, in_=ssum)
    nc.vector.tensor_scalar(
        out=comb[:, 0, :].bitcast(f32),
        in0=sig,
        scalar1=rec[:, 0:1],
        scalar2=ROUTE_SCALE,
        op0=mybir.AluOpType.mult,
        op1=mybir.AluOpType.mult,
    )
    # single merged output DMA (weights bitcast alongside indices); it
    # rides the SWDGE ring so the tiny writes never stall the HWDGE
    # rings that stream x and w; the tail blocks ride the (by then
    # idle) sync ring instead, which drains faster
    eng = out_eng if out_eng is not None else nc.gpsimd
    eng.dma_start(out=oout[t0:t0 + P, :, :], in_=comb)


def _build_fp16fp8():
    nc = bacc.Bacc("TRN2", target_bir_lowering=False, debug=False, num_devices=N_CORES)
    f32 = mybir.dt.float32
    f16 = mybir.dt.float16
    f8 = mybir.dt.float8e4
    # partition-major contiguous host layouts (28.7 KB runs per partition)
    xh = nc.dram_tensor("xh", [P, NB, KC, TB], f16, kind="ExternalInput").ap()
    xl8 = nc.dram_tensor("xl8", [P, NB, KC, TB], f8, kind="ExternalInput").ap()
    wh = nc.dram_tensor("wh", [P, KC, E], f16, kind="ExternalInput").ap()
    wl8d = nc.dram_tensor("wl8", [P, KC, E], f8, kind="ExternalInput").ap()
    oout = nc.dram_tensor(
        "o_out", [T, 2, TOPK], mybir.dt.uint32, kind="ExternalOutput"
    ).ap()

    with tile.TileContext(nc) as tc, ExitStack() as ctx:
        wh_pool = ctx.enter_context(tc.tile_pool(name="wh", bufs=1))
        w8_pool = ctx.enter_context(tc.tile_pool(name="w8", bufs=1))
        xh_pool = ctx.enter_context(tc.tile_pool(name="xh", bufs=2))
        x8_pool = ctx.enter_context(tc.tile_pool(name="x8", bufs=2))
        psA_pool = ctx.enter_context(tc.tile_pool(name="psA", bufs=4, space="PSUM"))
        psB_pool = ctx.enter_context(tc.tile_pool(name="psB", bufs=3, space="PSUM"))
        warm_pool = ctx.enter_context(tc.tile_pool(name="warmps", bufs=1, space="PSUM"))
        sc_pool = ctx.enter_context(tc.tile_pool(name="scratch", bufs=3))
        out_pool = ctx.enter_context(tc.tile_pool(name="outs", bufs=4))

        # PE p-state warmup: ~60 dummy matmuls on a memset tile fill the
        # DMA-bound head so the DVFS ramp (3us of busy to reach 2.4GHz)
        # is absorbed before real work arrives.
        warm = sc_pool.tile([P, P], f16, tag="warm")
        nc.vector.memset(warm, 0.0)
        wpsum = warm_pool.tile([P, P], mybir.dt.float32)
        for _ in range(60):
            nc.tensor.matmul(wpsum, warm, warm, start=True, stop=True)

        # The head is aggregate-HBM-bandwidth-bound, so spread the wh
        # quarters across BOTH HWDGE rings interleaved with block-0 x so
        # main(0) is never paced by a single slow ring.  wl8 + the x
        # residuals ride the Activation ring; wh8 = fp8(wh*2^-11) is an
        # on-device cast.
        wh_sb = [
            wh_pool.tile([P, WCQ, E], f16, tag=f"wh{q}", name=f"wh{q}")
            for q in range(WQ)
        ]
        xh0_tiles = []
        for q in range(KQ):
            nc.sync.dma_start(
                out=wh_sb[2 * q], in_=wh[:, 2 * q * WCQ:(2 * q + 1) * WCQ, :]
            )
            nc.scalar.dma_start(
                out=wh_sb[2 * q + 1],
                in_=wh[:, (2 * q + 1) * WCQ:(2 * q + 2) * WCQ, :],
            )
            t_ = xh_pool.tile([P, KCQ, TB], f16, tag=f"xh{q}")
            nc.sync.dma_start(out=t_, in_=xh[:, 0, q * KCQ:(q + 1) * KCQ, :])
            xh0_tiles.append(t_)

        w8_sb = []
        for q in range(WQ):
            w8t = w8_pool.tile([P, 2, WCQ, E], f8, tag=f"w8{q}")
            nc.scalar.dma_start(
                out=w8t[:, 0], in_=wl8d[:, q * WCQ:(q + 1) * WCQ, :]
            )
            if q % 2 == 0:
                nc.scalar.activation(
                    out=w8t[:, 1], in_=wh_sb[q],
                    func=mybir.ActivationFunctionType.Copy,
                    scale=WH8_DESCALE,
                )
            else:
                nc.vector.tensor_scalar_mul(
                    out=w8t[:, 1], in0=wh_sb[q], scalar1=WH8_DESCALE
                )
            w8_sb.append(w8t)

        def load_xh(b):
            tiles = []
            for q in range(KQ):
                t_ = xh_pool.tile([P, KCQ, TB], f16, tag=f"xh{q}")
                nc.sync.dma_start(
                    out=t_, in_=xh[:, b, q * KCQ:(q + 1) * KCQ, :]
                )
                tiles.append(t_)
            return tiles

        def load_x8(b, xh_tiles):
            # x8 pair tile [P, 2, KCQ, TB]: [:,0]=fp8(xh) cast on-device
            # (quarters alternate scalar/DVE so neither engine saturates),
            # [:,1]=xl8 DMA'd from host on the Activation ring (block 0's
            # rides the faster SP ring — it gates the first correction).
            xl8_eng = nc.sync if b == 0 else nc.scalar
            tiles = []
            for q in range(KQ):
                t_ = x8_pool.tile([P, 2, KCQ, TB], f8, tag=f"x8{q}")
                xl8_eng.dma_start(
                    out=t_[:, 1], in_=xl8[:, b, q * KCQ:(q + 1) * KCQ, :]
                )
                if q % 2 == 0:
                    nc.scalar.activation(
                        out=t_[:, 0], in_=xh_tiles[q],
                        func=mybir.ActivationFunctionType.Copy,
                    )
                else:
                    nc.vector.tensor_copy(out=t_[:, 0], in_=xh_tiles[q])
                tiles.append(t_)
            return tiles

        xh_tiles = {0: xh0_tiles}
        x8_tiles = {0: load_x8(0, xh0_tiles)}
        xh_tiles[1] = load_xh(1)

        for b in range(NB):
            xh_q = xh_tiles.pop(b)
            x8_q = x8_tiles.pop(b)
            # two CLOSED accumulation groups per tile (open/merged groups
            # measured +53 cycles on every matmul); main and correction are
            # both at scale 2^17 so the combine is a single tensor_add
            psA_list = []
            for j in range(TPB):
                js = slice(j * P, (j + 1) * P)
                psumA = psA_pool.tile([P, E], f32)
                for k in range(KC):
                    nc.tensor.matmul(
                        psumA,
                        xh_q[k // KCQ][:, k % KCQ, js],
                        wh_sb[k // WCQ][:, k % WCQ, :],
                        start=(k == 0),
                        stop=(k == KC - 1),
                    )
                psA_list.append(psumA)
            for j in range(TPB):
                js = slice(j * P, (j + 1) * P)
                psumB = psB_pool.tile([P, E], f32)
                for k in range(KC):
                    nc.tensor.matmul(
                        psumB,
                        x8_q[k // KCQ][:, :, k % KCQ, js],
                        w8_sb[k // WCQ][:, :, k % WCQ, :],
                        start=(k == 0),
                        stop=(k == KC - 1),
                        perf_mode=mybir.MatmulPerfMode.DoubleRow,
                    )
                scores = sc_pool.tile([P, E], f32)
                nc.scalar.activation(
                    out=scores, in_=psumB,
                    func=mybir.ActivationFunctionType.Copy,
                )
                nc.vector.tensor_add(scores, scores, psA_list[j])
                _emit_topk(
                    nc, sc_pool, out_pool, scores, oout, b * TB + j * P,
                    sig_scale=1.0 / W_SCALE,
                    out_eng=nc.sync if b >= NB - 2 else None,
                )
            # next block's loads AFTER this block's epilogues: keeps the
            # scalar/DVE queues draining the topk (and releasing PSUM)
            # before they start the next casts
            if b + 2 < NB:
                xh_tiles[b + 2] = load_xh(b + 2)
            if b + 1 < NB:
                x8_tiles[b + 1] = load_x8(b + 1, xh_tiles[b + 1])
    nc.compile()
    return nc


def _build_fp16x3():
    """Baseline 3-pass fp16 splitting kernel (fallback)."""
    nc = bacc.Bacc("TRN2", target_bir_lowering=False, debug=False, num_devices=N_CORES)
    f32 = mybir.dt.float32
    f16 = mybir.dt.float16
    xh = nc.dram_tensor("xh", [D, T], f16, kind="ExternalInput").ap()
    xl = nc.dram_tensor("xl", [D, T], f16, kind="ExternalInput").ap()
    wh = nc.dram_tensor("wh", [D, E], f16, kind="ExternalInput").ap()
    wl = nc.dram_tensor("wl", [D, E], f16, kind="ExternalInput").ap()
    oout = nc.dram_tensor(
        "o_out", [T, 2, TOPK], mybir.dt.uint32, kind="ExternalOutput"
    ).ap()

    xh_r = xh.rearrange("(k p) t -> p k t", p=P)
    xl_r = xl.rearrange("(k p) t -> p k t", p=P)
    wh_r = wh.rearrange("(k p) e -> p k e", p=P)
    wl_r = wl.rearrange("(k p) e -> p k e", p=P)

    with tile.TileContext(nc) as tc, ExitStack() as ctx:
        wt_pool = ctx.enter_context(tc.tile_pool(name="wt", bufs=1))
        xt_pool = ctx.enter_context(tc.tile_pool(name="xt", bufs=2))
        psA_pool = ctx.enter_context(tc.tile_pool(name="psA", bufs=4, space="PSUM"))
        psB_pool = ctx.enter_context(tc.tile_pool(name="psB", bufs=4, space="PSUM"))
        sc_pool = ctx.enter_context(tc.tile_pool(name="scratch", bufs=3))
        out_pool = ctx.enter_context(tc.tile_pool(name="outs", bufs=4))

        def load_w(q, which):
            src, lst, tag = (
                (wh_r, wh_sb, f"wh{q}") if which == "h" else (wl_r, wl_sb, f"wl{q}")
            )
            wtile = wt_pool.tile([P, WCQ, E], f16, tag=tag)
            nc.sync.dma_start(out=wtile, in_=src[:, q * WCQ:(q + 1) * WCQ, :])
            lst.append(wtile)

        def load_x_block(b):
            xh_q, xl_q = [], []
            t_lo, t_hi = b * TB, (b + 1) * TB
            for q in range(KQ):
                xtile = xt_pool.tile([P, KCQ, TB], f16, tag=f"xh{q}")
                nc.sync.dma_start(
                    out=xtile, in_=xh_r[:, q * KCQ:(q + 1) * KCQ, t_lo:t_hi]
                )
                xh_q.append(xtile)
                ltile = xt_pool.tile([P, KCQ, TB], f16, tag=f"xl{q}")
                nc.sync.dma_start(
                    out=ltile, in_=xl_r[:, q * KCQ:(q + 1) * KCQ, t_lo:t_hi]
                )
                xl_q.append(ltile)
            return xh_q, xl_q

        wh_sb, wl_sb = [], []
        xh0, xl0 = [], []
        t_hi0 = TB
        for q in range(KQ):
            load_w(2 * q, "h")
            load_w(2 * q + 1, "h")
            xtile = xt_pool.tile([P, KCQ, TB], f16, tag=f"xh{q}")
            nc.sync.dma_start(out=xtile, in_=xh_r[:, q * KCQ:(q + 1) * KCQ, 0:t_hi0])
            xh0.append(xtile)
        for q in range(KQ):
            ltile = xt_pool.tile([P, KCQ, TB], f16, tag=f"xl{q}")
            nc.sync.dma_start(out=ltile, in_=xl_r[:, q * KCQ:(q + 1) * KCQ, 0:t_hi0])
            xl0.append(ltile)
        for q in range(WQ):
            load_w(q, "l")
        blocks = {0: (xh0, xl0)}

        def flush(state):
            bb, xh_q, psA_list, psB_list = state
            for j in range(TPB):
                js = slice(j * P, (j + 1) * P)
                psumB = psB_list[j]
                for k in range(KC):
                    nc.tensor.matmul(
                        psumB,
                        xh_q[k // KCQ][:, k % KCQ, js],
                        wl_sb[k // WCQ][:, k % WCQ, :],
                        start=False,
                        stop=(k == KC - 1),
                    )
                scores = sc_pool.tile([P, E], f32)
                nc.scalar.activation(
                    out=scores,
                    in_=psumB,
                    func=mybir.ActivationFunctionType.Copy,
                    scale=1.0 / XL_SCALE,
                )
                nc.vector.tensor_add(scores, scores, psA_list[j])
                _emit_topk(nc, sc_pool, out_pool, scores, oout, bb * TB + j * P)

        pending = None
        for b in range(NB):
            if b not in blocks:
                blocks[b] = load_x_block(b)
            xh_q, xl_q = blocks.pop(b)
            if b == 0:
                psA_list, psB_list = [], []
                for j in range(TPB):
                    js = slice(j * P, (j + 1) * P)
                    psumA = psA_pool.tile([P, E], f32)
                    for k in range(KC):
                        nc.tensor.matmul(
                            psumA,
                            xh_q[k // KCQ][:, k % KCQ, js],
                            wh_sb[k // WCQ][:, k % WCQ, :],
                            start=(k == 0),
                            stop=(k == KC - 1),
                        )
                    psA_list.append(psumA)
                for j in range(TPB):
                    js = slice(j * P, (j + 1) * P)
                    psumB = psB_pool.tile([P, E], f32)
                    for k in range(KC):
                        nc.tensor.matmul(
                            psumB,
                            xl_q[k // KCQ][:, k % KCQ, js],
                            wh_sb[k // WCQ][:, k % WCQ, :],
                            start=(k == 0),
                            stop=False,
                        )
                    psB_list.append(psumB)
                pending = (b, xh_q, psA_list, psB_list)
                continue
            for j in range(TPB):
                js = slice(j * P, (j + 1) * P)
                psumA = psA_pool.tile([P, E], f32)
                for k in range(KC):
                    nc.tensor.matmul(
                        psumA,
                        xh_q[k // KCQ][:, k % KCQ, js],
                        wh_sb[k // WCQ][:, k % WCQ, :],
                        start=(k == 0),
                        stop=(k == KC - 1),
                    )
                if pending is not None:
                    flush(pending)
                    pending = None
                psumB = psB_pool.tile([P, E], f32)
                for i in range(2 * KC):
                    k = i % KC
                    if i < KC:
                        lhsT = xl_q[k // KCQ][:, k % KCQ, js]
                        rhs = wh_sb[k // WCQ][:, k % WCQ, :]
                    else:
                        lhsT = xh_q[k // KCQ][:, k % KCQ, js]
                        rhs = wl_sb[k // WCQ][:, k % WCQ, :]
                    nc.tensor.matmul(
                        psumB, lhsT, rhs, start=(i == 0), stop=(i == 2 * KC - 1)
                    )
                scores = sc_pool.tile([P, E], f32)
                nc.scalar.activation(
                    out=scores,
                    in_=psumB,
                    func=mybir.ActivationFunctionType.Copy,
                    scale=1.0 / XL_SCALE,
                )
                nc.vector.tensor_add(scores, scores, psumA)
                _emit_topk(nc, sc_pool, out_pool, scores, oout, b * TB + j * P)
    nc.compile()
    return nc


def _get_program(precision):
    key = f"nc_{precision}"
    if key not in _CACHE:
        _CACHE[key] = (
            _build_fp16fp8() if precision == "fp16fp8" else _build_fp16x3()
        )
    return _CACHE[key]


def _pack_x_block_major(a, c):
    """[T_FULL, D] core-c slice -> [P, NB, KC, TB] contiguous."""
    s = a[c * T:(c + 1) * T, :]
    return np.ascontiguousarray(
        s.reshape(NB, TB, KC, P).transpose(3, 0, 2, 1)
    )


def _pack_w(a):
    """[E, D] -> [P, KC, E] contiguous."""
    return np.ascontiguousarray(a.reshape(E, KC, P).transpose(2, 1, 0))


def kernel(x: np.ndarray, weight: np.ndarray, _trace: bool = False, **_kw):
    x = np.asarray(x, dtype=np.float32)
    weight = np.asarray(weight, dtype=np.float32)
    assert x.shape == (T_FULL, D) and weight.shape == (E, D)

    nc = _get_program(PRECISION)
    if PRECISION == "fp16fp8":
        xh16 = x.astype(np.float16)
        xl8_full = ((x - xh16.astype(np.float32)) * np.float32(XL_SCALE)).astype(E4)
        ws = weight * np.float32(W_SCALE)
        wh16 = ws.astype(np.float16)
        wl8 = (ws - wh16.astype(np.float32)).astype(E4)
        wh_host = _pack_w(wh16)
        wl8_host = _pack_w(wl8)
        in_maps = [
            {
                "xh": _pack_x_block_major(xh16, c),
                "xl8": _pack_x_block_major(xl8_full, c),
                "wh": wh_host,
                "wl8": wl8_host,
            }
            for c in range(N_CORES)
        ]
    else:
        xt_full = np.ascontiguousarray(x.T)
        wt_host = np.ascontiguousarray(weight.T)
        xh_f, xl_f = xt_full.astype(np.float16), None
        xl_f = ((xt_full - xh_f.astype(np.float32)) * np.float32(XL_SCALE)).astype(
            np.float16
        )
        whh = wt_host.astype(np.float16)
        wll = ((wt_host - whh.astype(np.float32)) * np.float32(XL_SCALE)).astype(
            np.float16
        )
        in_maps = [
            {
                "xh": np.ascontiguousarray(xh_f[:, c * T:(c + 1) * T]),
                "xl": np.ascontiguousarray(xl_f[:, c * T:(c + 1) * T]),
                "wh": whh,
                "wl": wll,
            }
            for c in range(N_CORES)
        ]
    if _trace:
        import tempfile

        res = run_bass_kernel_spmd(
            nc, in_maps, core_ids=list(range(N_CORES)), trace=True,
            tmpdir=tempfile.mkdtemp(prefix="moe_gate_trace_"),
        )
        results = res.results
        _CACHE["last_result"] = {
            "exec_time_ns": res.exec_time_ns,
            "percore": res.mean_exec_time_ns,
            "neff_dir": res.instructions_and_trace[1]
            if res.instructions_and_trace
            else None,
        }
    else:
        res = run_bass_kernel_spmd(nc, in_maps, core_ids=list(range(N_CORES)))
        results = res.results
    o_full = np.concatenate(
        [np.asarray(results[c]["o_out"]) for c in range(N_CORES)], axis=0
    )  # [T_FULL, 2, TOPK] u32: pair 0 = weights (f32 bits), pair 1 = indices
    w_full = np.ascontiguousarray(o_full[:, 0, :]).view(np.float32)
    i_full = np.ascontiguousarray(o_full[:, 1, :]).astype(np.int32)
    return w_full, i_full


# revision 35
# speedup vs baseline: 1.0170x; 1.0170x over previous
"""MoE group-limited routing gate (DeepSeek-style) on 8 Trainium2 NeuronCores.

Computation (per token t over E=256 experts, D=7168 features):
    logits = x @ weight.T                      [T, E]
    group-limited top-k: 8 groups of 32 experts, keep top-4 groups by
    group-max, then top-8 experts among kept groups.
    weights = sigmoid(logits[sel]) normalized to sum 1, * 2.5
Returns (weights [T,8] f32, indices [T,8] int32) like the reference.

Strategy: data-parallel over tokens, 2048 tokens/core, gate weight
replicated.  Matmul precision "fp16fp8" (scores kept at scale 2^17;
the 2^-17 descale folds into the sigmoid's scale argument):
    2^17 * logits = xh16 @ wh16  +  (x8 @ wl8 + xl8 @ wh8)
  - main pass: fp16 (11-bit significands, products exact in f32 PSUM);
    wh16 = fp16(w * 2^17) (fits fp16 range, kills w subnormals)
  - correction: ONE fp8e4m3 DoubleRow pass fusing both residual terms:
    x8 = fp8(xh16) and wh8 = fp8(wh16 * 2^-11), both cast on-device;
    xl8 = fp8((x - xh16) * 2^11) and wl8 = fp8(w*2^17 - wh16) from
    host.  DoubleRow runs fp8 at 2x rate, so the whole correction
    costs one bf16-rate pass -> 2 pass-equivalents vs fp16x3's 3.
  Host-measured logit err ~1.8e-5 -> idx rel-err ~6e-3 on the graded
  inputs (vs 2e-2 gate), identical on hardware (exact fp16/fp8
  products in f32 PSUM).
DMA: x at 3 B/elem in partition-major contiguous blocks: xh16 on the
SP HWDGE ring, xl8 on the Activation ring (block 0's on SP - it gates
the first correction); wh quarters alternate across BOTH rings
interleaved with block-0 x (the head is aggregate-HBM-bound).
Outputs merge weights (f32 bitcast) + indices into one u32 DMA per
token-tile on the SWDGE ring; tail blocks use the by-then-idle SP
ring.  Accumulation groups stay CLOSED per pass: leaving them open
across the fp16+fp8 passes measured +53 cycles on EVERY matmul.
Top-k uses the DVE native max/max_index (top-8 sorted) instructions;
the group top-4 uses a threshold trick (4th-largest group-max) since
sigmoid is monotone and masking is additive on logits.
"""

import numpy as np
import ml_dtypes
from contextlib import ExitStack

import concourse.bacc as bacc
import concourse.tile as tile
from concourse import mybir
from concourse.bass_utils import run_bass_kernel_spmd

N_CORES = 8
T_FULL = 16384
D = 7168
E = 256
G = 8            # expert groups
EPG = E // G     # experts per group = 32
TOPK = 8
TOPK_GROUPS = 4
ROUTE_SCALE = 2.5

P = 128
T = T_FULL // N_CORES       # 2048 tokens per core
KC = D // P                 # 56 contraction chunks
TB = 256                    # tokens per block
NB = T // TB                # 8 blocks
TPB = TB // P               # 2 token-tiles per block
KQ = 4                      # x DMA splits per block
KCQ = KC // KQ              # 14 k-chunks per split
WQ = 8                      # weight DMA splits
WCQ = KC // WQ              # 7 k-chunks per split
NEG = -1.0e30
XL_SCALE = 2.0 ** 11        # xl8 = fp8((x - xh16) * XL_SCALE)
W_SCALE = 2.0 ** 17         # wh16 stores w * 2^17 (fits fp16, no subnormals).
                            # Main (xh@wh16) and correction (x8@wl8 + xl8@wh8)
                            # are then BOTH at scale 2^17, so the combine is a
                            # plain copy+add and 2^-17 folds into the sigmoid.
WH8_DESCALE = 1.0 / XL_SCALE  # wh8 = fp8(wh16 * 2^-11), on-device cast
E4 = ml_dtypes.float8_e4m3
PRECISION = "fp16fp8"       # "fp16fp8" | "fp16x3"

_CACHE = {}


def _emit_topk(nc, sc_pool, out_pool, scores, oout, t0,
               sig_scale=1.0, out_eng=None):
    """Group-limited top-k + normalize on a [128, 256] f32 logits tile.

    ``scores`` may be pre-scaled logits (monotone, so group-mask and top-k
    are unaffected); ``sig_scale`` restores true logits inside the sigmoid.
    Writes one merged [P, 2, TOPK] u32 output row-block: pair 0 = weights
    (f32 bitcast), pair 1 = indices.
    """
    f32 = mybir.dt.float32
    scores_g = scores.rearrange("p (g e) -> p g e", g=G)
    glog = sc_pool.tile([P, G], f32)
    nc.vector.reduce_max(out=glog, in_=scores_g, axis=mybir.AxisListType.X)
    gsort = sc_pool.tile([P, G], f32)
    nc.vector.max(out=gsort, in_=glog)
    # additive mask: 0 for kept groups (>= 4th-largest), -1e30 otherwise
    maskadd = sc_pool.tile([P, G], f32)
    nc.vector.tensor_scalar(
        out=maskadd,
        in0=glog,
        scalar1=gsort[:, TOPK_GROUPS - 1:TOPK_GROUPS],
        scalar2=NEG,
        op0=mybir.AluOpType.is_lt,
        op1=mybir.AluOpType.mult,
    )
    masked = sc_pool.tile([P, E], f32)
    nc.vector.tensor_add(
        masked.rearrange("p (g e) -> p g e", g=G),
        scores_g,
        maskadd.to_broadcast([P, G, EPG]),
    )
    top8 = sc_pool.tile([P, TOPK], f32)
    nc.vector.max(out=top8, in_=masked)
    comb = out_pool.tile([P, 2, TOPK], mybir.dt.uint32)
    nc.vector.max_index(out=comb[:, 1, :], in_max=top8, in_values=masked)
    sig = sc_pool.tile([P, TOPK], f32)
    nc.scalar.activation(
        out=sig, in_=top8, func=mybir.ActivationFunctionType.Sigmoid,
        scale=sig_scale,
    )
    ssum = sc_pool.tile([P, 1], f32)
    nc.vector.reduce_sum(out=ssum, in_=sig, axis=mybir.AxisListType.X)
    rec = sc_pool.tile([P, 1], f32)
    nc.vector.reciprocal(out=rec, in_=ssum)
    nc.vector.tensor_scalar(
        out=comb[:, 0, :].bitcast(f32),
        in0=sig,
        scalar1=rec[:, 0:1],
        scalar2=ROUTE_SCALE,
        op0=mybir.AluOpType.mult,
        op1=mybir.AluOpType.mult,
    )
    # single merged output DMA (weights bitcast alongside indices); it
    # rides the SWDGE ring so the tiny writes never stall the HWDGE
    # rings that stream x and w; the tail blocks ride the (by then
    # idle) sync ring instead, which drains faster
    eng = out_eng if out_eng is not None else nc.gpsimd
    eng.dma_start(out=oout[t0:t0 + P, :, :], in_=comb)


def _build_fp16fp8():
    nc = bacc.Bacc("TRN2", target_bir_lowering=False, debug=False, num_devices=N_CORES)
    f32 = mybir.dt.float32
    f16 = mybir.dt.float16
    f8 = mybir.dt.float8e4
    # partition-major contiguous host layouts (28.7 KB runs per partition)
    xh = nc.dram_tensor("xh", [P, NB, KC, TB], f16, kind="ExternalInput").ap()
    xl8 = nc.dram_tensor("xl8", [P, NB, KC, TB], f8, kind="ExternalInput").ap()
    wh = nc.dram_tensor("wh", [P, KC, E], f16, kind="ExternalInput").ap()
    wl8d = nc.dram_tensor("wl8", [P, KC, E], f8, kind="ExternalInput").ap()
    oout = nc.dram_tensor(
        "o_out", [T, 2, TOPK], mybir.dt.uint32, kind="ExternalOutput"
    ).ap()

    with tile.TileContext(nc) as tc, ExitStack() as ctx:
        wh_pool = ctx.enter_context(tc.tile_pool(name="wh", bufs=1))
        w8_pool = ctx.enter_context(tc.tile_pool(name="w8", bufs=1))
        xh_pool = ctx.enter_context(tc.tile_pool(name="xh", bufs=2))
        x8_pool = ctx.enter_context(tc.tile_pool(name="x8", bufs=2))
        psA_pool = ctx.enter_context(tc.tile_pool(name="psA", bufs=4, space="PSUM"))
        psB_pool = ctx.enter_context(tc.tile_pool(name="psB", bufs=4, space="PSUM"))
        sc_pool = ctx.enter_context(tc.tile_pool(name="scratch", bufs=3))
        out_pool = ctx.enter_context(tc.tile_pool(name="outs", bufs=4))

        # The head is aggregate-HBM-bandwidth-bound, so spread the wh
        # quarters across BOTH HWDGE rings interleaved with block-0 x so
        # main(0) is never paced by a single slow ring.  wl8 + the x
        # residuals ride the Activation ring; wh8 = fp8(wh*2^-11) is an
        # on-device cast.
        wh_sb = [
            wh_pool.tile([P, WCQ, E], f16, tag=f"wh{q}", name=f"wh{q}")
            for q in range(WQ)
        ]
        xh0_tiles = []
        for q in range(KQ):
            nc.sync.dma_start(
                out=wh_sb[2 * q], in_=wh[:, 2 * q * WCQ:(2 * q + 1) * WCQ, :]
            )
            nc.scalar.dma_start(
                out=wh_sb[2 * q + 1],
                in_=wh[:, (2 * q + 1) * WCQ:(2 * q + 2) * WCQ, :],
            )
            t_ = xh_pool.tile([P, KCQ, TB], f16, tag=f"xh{q}")
            nc.sync.dma_start(out=t_, in_=xh[:, 0, q * KCQ:(q + 1) * KCQ, :])
            xh0_tiles.append(t_)

        w8_sb = []
        for q in range(WQ):
            w8t = w8_pool.tile([P, 2, WCQ, E], f8, tag=f"w8{q}")
            nc.scalar.dma_start(
                out=w8t[:, 0], in_=wl8d[:, q * WCQ:(q + 1) * WCQ, :]
            )
            if q % 2 == 0:
                nc.scalar.activation(
                    out=w8t[:, 1], in_=wh_sb[q],
                    func=mybir.ActivationFunctionType.Copy,
                    scale=WH8_DESCALE,
                )
            else:
                nc.vector.tensor_scalar_mul(
                    out=w8t[:, 1], in0=wh_sb[q], scalar1=WH8_DESCALE
                )
            w8_sb.append(w8t)

        def load_xh(b):
            tiles = []
            for q in range(KQ):
                t_ = xh_pool.tile([P, KCQ, TB], f16, tag=f"xh{q}")
                nc.sync.dma_start(
                    out=t_, in_=xh[:, b, q * KCQ:(q + 1) * KCQ, :]
                )
                tiles.append(t_)
            return tiles

        def load_x8(b, xh_tiles):
            # x8 pair tile [P, 2, KCQ, TB]: [:,0]=fp8(xh) cast on-device
            # (quarters alternate scalar/DVE so neither engine saturates),
            # [:,1]=xl8 DMA'd from host on the Activation ring (block 0's
            # rides the faster SP ring — it gates the first correction).
            xl8_eng = nc.sync if b == 0 else nc.scalar
            tiles = []
            for q in range(KQ):
                t_ = x8_pool.tile([P, 2, KCQ, TB], f8, tag=f"x8{q}")
                xl8_eng.dma_start(
                    out=t_[:, 1], in_=xl8[:, b, q * KCQ:(q + 1) * KCQ, :]
                )
                if q % 2 == 0:
                    nc.scalar.activation(
                        out=t_[:, 0], in_=xh_tiles[q],
                        func=mybir.ActivationFunctionType.Copy,
                    )
                else:
                    nc.vector.tensor_copy(out=t_[:, 0], in_=xh_tiles[q])
                tiles.append(t_)
            return tiles

        xh_tiles = {0: xh0_tiles}
        x8_tiles = {0: load_x8(0, xh0_tiles)}
        xh_tiles[1] = load_xh(1)

        for b in range(NB):
            xh_q = xh_tiles.pop(b)
            x8_q = x8_tiles.pop(b)
            # two CLOSED accumulation groups per tile (open/merged groups
            # measured +53 cycles on every matmul); main and correction are
            # both at scale 2^17 so the combine is a single tensor_add
            psA_list = []
            for j in range(TPB):
                js = slice(j * P, (j + 1) * P)
                psumA = psA_pool.tile([P, E], f32)
                for k in range(KC):
                    nc.tensor.matmul(
                        psumA,
                        xh_q[k // KCQ][:, k % KCQ, js],
                        wh_sb[k // WCQ][:, k % WCQ, :],
                        start=(k == 0),
                        stop=(k == KC - 1),
                    )
                psA_list.append(psumA)
            for j in range(TPB):
                js = slice(j * P, (j + 1) * P)
                psumB = psB_pool.tile([P, E], f32)
                for k in range(KC):
                    nc.tensor.matmul(
                        psumB,
                        x8_q[k // KCQ][:, :, k % KCQ, js],
                        w8_sb[k // WCQ][:, :, k % WCQ, :],
                        start=(k == 0),
                        stop=(k == KC - 1),
                        perf_mode=mybir.MatmulPerfMode.DoubleRow,
                    )
                scores = sc_pool.tile([P, E], f32)
                nc.scalar.activation(
                    out=scores, in_=psumB,
                    func=mybir.ActivationFunctionType.Copy,
                )
                nc.vector.tensor_add(scores, scores, psA_list[j])
                _emit_topk(
                    nc, sc_pool, out_pool, scores, oout, b * TB + j * P,
                    sig_scale=1.0 / W_SCALE,
                    out_eng=nc.sync if b >= NB - 2 else None,
                )
            # next block's loads AFTER this block's epilogues: keeps the
            # scalar/DVE queues draining the topk (and releasing PSUM)
            # before they start the next casts
            if b + 2 < NB:
                xh_tiles[b + 2] = load_xh(b + 2)
            if b + 1 < NB:
                x8_tiles[b + 1] = load_x8(b + 1, xh_tiles[b + 1])
    nc.compile()
    return nc


def _build_fp16x3():
    """Baseline 3-pass fp16 splitting kernel (fallback)."""
    nc = bacc.Bacc("TRN2", target_bir_lowering=False, debug=False, num_devices=N_CORES)
    f32 = mybir.dt.float32
    f16 = mybir.dt.float16
    xh = nc.dram_tensor("xh", [D, T], f16, kind="ExternalInput").ap()
    xl = nc.dram_tensor("xl", [D, T], f16, kind="ExternalInput").ap()
    wh = nc.dram_tensor("wh", [D, E], f16, kind="ExternalInput").ap()
    wl = nc.dram_tensor("wl", [D, E], f16, kind="ExternalInput").ap()
    oout = nc.dram_tensor(
        "o_out", [T, 2, TOPK], mybir.dt.uint32, kind="ExternalOutput"
    ).ap()

    xh_r = xh.rearrange("(k p) t -> p k t", p=P)
    xl_r = xl.rearrange("(k p) t -> p k t", p=P)
    wh_r = wh.rearrange("(k p) e -> p k e", p=P)
    wl_r = wl.rearrange("(k p) e -> p k e", p=P)

    with tile.TileContext(nc) as tc, ExitStack() as ctx:
        wt_pool = ctx.enter_context(tc.tile_pool(name="wt", bufs=1))
        xt_pool = ctx.enter_context(tc.tile_pool(name="xt", bufs=2))
        psA_pool = ctx.enter_context(tc.tile_pool(name="psA", bufs=4, space="PSUM"))
        psB_pool = ctx.enter_context(tc.tile_pool(name="psB", bufs=4, space="PSUM"))
        sc_pool = ctx.enter_context(tc.tile_pool(name="scratch", bufs=3))
        out_pool = ctx.enter_context(tc.tile_pool(name="outs", bufs=4))

        def load_w(q, which):
            src, lst, tag = (
                (wh_r, wh_sb, f"wh{q}") if which == "h" else (wl_r, wl_sb, f"wl{q}")
            )
            wtile = wt_pool.tile([P, WCQ, E], f16, tag=tag)
            nc.sync.dma_start(out=wtile, in_=src[:, q * WCQ:(q + 1) * WCQ, :])
            lst.append(wtile)

        def load_x_block(b):
            xh_q, xl_q = [], []
            t_lo, t_hi = b * TB, (b + 1) * TB
            for q in range(KQ):
                xtile = xt_pool.tile([P, KCQ, TB], f16, tag=f"xh{q}")
                nc.sync.dma_start(
                    out=xtile, in_=xh_r[:, q * KCQ:(q + 1) * KCQ, t_lo:t_hi]
                )
                xh_q.append(xtile)
                ltile = xt_pool.tile([P, KCQ, TB], f16, tag=f"xl{q}")
                nc.sync.dma_start(
                    out=ltile, in_=xl_r[:, q * KCQ:(q + 1) * KCQ, t_lo:t_hi]
                )
                xl_q.append(ltile)
            return xh_q, xl_q

        wh_sb, wl_sb = [], []
        xh0, xl0 = [], []
        t_hi0 = TB
        for q in range(KQ):
            load_w(2 * q, "h")
            load_w(2 * q + 1, "h")
            xtile = xt_pool.tile([P, KCQ, TB], f16, tag=f"xh{q}")
            nc.sync.dma_start(out=xtile, in_=xh_r[:, q * KCQ:(q + 1) * KCQ, 0:t_hi0])
            xh0.append(xtile)
        for q in range(KQ):
            ltile = xt_pool.tile([P, KCQ, TB], f16, tag=f"xl{q}")
            nc.sync.dma_start(out=ltile, in_=xl_r[:, q * KCQ:(q + 1) * KCQ, 0:t_hi0])
            xl0.append(ltile)
        for q in range(WQ):
            load_w(q, "l")
        blocks = {0: (xh0, xl0)}

        def flush(state):
            bb, xh_q, psA_list, psB_list = state
            for j in range(TPB):
                js = slice(j * P, (j + 1) * P)
                psumB = psB_list[j]
                for k in range(KC):
                    nc.tensor.matmul(
                        psumB,
                        xh_q[k // KCQ][:, k % KCQ, js],
                        wl_sb[k // WCQ][:, k % WCQ, :],
                        start=False,
                        stop=(k == KC - 1),
                    )
                scores = sc_pool.tile([P, E], f32)
                nc.scalar.activation(
                    out=scores,
                    in_=psumB,
                    func=mybir.ActivationFunctionType.Copy,
                    scale=1.0 / XL_SCALE,
                )
                nc.vector.tensor_add(scores, scores, psA_list[j])
                _emit_topk(nc, sc_pool, out_pool, scores, oout, bb * TB + j * P)

        pending = None
        for b in range(NB):
            if b not in blocks:
                blocks[b] = load_x_block(b)
            xh_q, xl_q = blocks.pop(b)
            if b == 0:
                psA_list, psB_list = [], []
                for j in range(TPB):
                    js = slice(j * P, (j + 1) * P)
                    psumA = psA_pool.tile([P, E], f32)
                    for k in range(KC):
                        nc.tensor.matmul(
                            psumA,
                            xh_q[k // KCQ][:, k % KCQ, js],
                            wh_sb[k // WCQ][:, k % WCQ, :],
                            start=(k == 0),
                            stop=(k == KC - 1),
                        )
                    psA_list.append(psumA)
                for j in range(TPB):
                    js = slice(j * P, (j + 1) * P)
                    psumB = psB_pool.tile([P, E], f32)
                    for k in range(KC):
                        nc.tensor.matmul(
                            psumB,
                            xl_q[k // KCQ][:, k % KCQ, js],
                            wh_sb[k // WCQ][:, k % WCQ, :],
                            start=(k == 0),
                            stop=False,
                        )
                    psB_list.append(psumB)
                pending = (b, xh_q, psA_list, psB_list)
                continue
            for j in range(TPB):
                js = slice(j * P, (j + 1) * P)
                psumA = psA_pool.tile([P, E], f32)
                for k in range(KC):
                    nc.tensor.matmul(
                        psumA,
                        xh_q[k // KCQ][:, k % KCQ, js],
                        wh_sb[k // WCQ][:, k % WCQ, :],
                        start=(k == 0),
                        stop=(k == KC - 1),
                    )
                if pending is not None:
                    flush(pending)
                    pending = None
                psumB = psB_pool.tile([P, E], f32)
                for i in range(2 * KC):
                    k = i % KC
                    if i < KC:
                        lhsT = xl_q[k // KCQ][:, k % KCQ, js]
                        rhs = wh_sb[k // WCQ][:, k % WCQ, :]
                    else:
                        lhsT = xh_q[k // KCQ][:, k % KCQ, js]
                        rhs = wl_sb[k // WCQ][:, k % WCQ, :]
                    nc.tensor.matmul(
                        psumB, lhsT, rhs, start=(i == 0), stop=(i == 2 * KC - 1)
                    )
                scores = sc_pool.tile([P, E], f32)
                nc.scalar.activation(
                    out=scores,
                    in_=psumB,
                    func=mybir.ActivationFunctionType.Copy,
                    scale=1.0 / XL_SCALE,
                )
                nc.vector.tensor_add(scores, scores, psumA)
                _emit_topk(nc, sc_pool, out_pool, scores, oout, b * TB + j * P)
    nc.compile()
    return nc


def _get_program(precision):
    key = f"nc_{precision}"
    if key not in _CACHE:
        _CACHE[key] = (
            _build_fp16fp8() if precision == "fp16fp8" else _build_fp16x3()
        )
    return _CACHE[key]


def _pack_x_block_major(a, c):
    """[T_FULL, D] core-c slice -> [P, NB, KC, TB] contiguous."""
    s = a[c * T:(c + 1) * T, :]
    return np.ascontiguousarray(
        s.reshape(NB, TB, KC, P).transpose(3, 0, 2, 1)
    )


def _pack_w(a):
    """[E, D] -> [P, KC, E] contiguous."""
    return np.ascontiguousarray(a.reshape(E, KC, P).transpose(2, 1, 0))


def kernel(x: np.ndarray, weight: np.ndarray, _trace: bool = False, **_kw):
    x = np.asarray(x, dtype=np.float32)
    weight = np.asarray(weight, dtype=np.float32)
    assert x.shape == (T_FULL, D) and weight.shape == (E, D)

    nc = _get_program(PRECISION)
    if PRECISION == "fp16fp8":
        xh16 = x.astype(np.float16)
        xl8_full = ((x - xh16.astype(np.float32)) * np.float32(XL_SCALE)).astype(E4)
        ws = weight * np.float32(W_SCALE)
        wh16 = ws.astype(np.float16)
        wl8 = (ws - wh16.astype(np.float32)).astype(E4)
        wh_host = _pack_w(wh16)
        wl8_host = _pack_w(wl8)
        in_maps = [
            {
                "xh": _pack_x_block_major(xh16, c),
                "xl8": _pack_x_block_major(xl8_full, c),
                "wh": wh_host,
                "wl8": wl8_host,
            }
            for c in range(N_CORES)
        ]
    else:
        xt_full = np.ascontiguousarray(x.T)
        wt_host = np.ascontiguousarray(weight.T)
        xh_f, xl_f = xt_full.astype(np.float16), None
        xl_f = ((xt_full - xh_f.astype(np.float32)) * np.float32(XL_SCALE)).astype(
            np.float16
        )
        whh = wt_host.astype(np.float16)
        wll = ((wt_host - whh.astype(np.float32)) * np.float32(XL_SCALE)).astype(
            np.float16
        )
        in_maps = [
            {
                "xh": np.ascontiguousarray(xh_f[:, c * T:(c + 1) * T]),
                "xl": np.ascontiguousarray(xl_f[:, c * T:(c + 1) * T]),
                "wh": whh,
                "wl": wll,
            }
            for c in range(N_CORES)
        ]
    if _trace:
        import tempfile

        res = run_bass_kernel_spmd(
            nc, in_maps, core_ids=list(range(N_CORES)), trace=True,
            tmpdir=tempfile.mkdtemp(prefix="moe_gate_trace_"),
        )
        results = res.results
        _CACHE["last_result"] = {
            "exec_time_ns": res.exec_time_ns,
            "percore": res.mean_exec_time_ns,
            "neff_dir": res.instructions_and_trace[1]
            if res.instructions_and_trace
            else None,
        }
    else:
        res = run_bass_kernel_spmd(nc, in_maps, core_ids=list(range(N_CORES)))
        results = res.results
    o_full = np.concatenate(
        [np.asarray(results[c]["o_out"]) for c in range(N_CORES)], axis=0
    )  # [T_FULL, 2, TOPK] u32: pair 0 = weights (f32 bits), pair 1 = indices
    w_full = np.ascontiguousarray(o_full[:, 0, :]).view(np.float32)
    i_full = np.ascontiguousarray(o_full[:, 1, :]).astype(np.int32)
    return w_full, i_full
